# revision 42
# baseline (speedup 1.0000x reference)
"""Distributed gaussian-mask attention for trn2 (8 NeuronCores, SPMD).

Problem: B=2, S=2048, H=1024, 16 heads, hd=64.
  q/k/v = x@W*, dif = q - k, score = exp(-0.5 * dif @ dif^T),
  prob = score * triu(ones,k=1), ctx = prob @ v, out = ctx @ Wo + bo.
  (bq/bk/bv are zeros by construction -- folded out; dif = x @ (Wq-Wk),
   and Wd = Wq-Wk is folded on the host.)

Sharding (uniform SPMD program, data-only per-core differences):
  - Head parallel: core c owns heads (2c, 2c+1) = 128 feature columns of
    Wq/Wk/Wv.  Each core computes D^T = x @ Wd_c and V for ALL tokens of
    its 2 heads, runs the full anti-causal attention triangle locally
    (no collective), producing ctx^T [128, 4096].
  - One AllToAll per batch (partition-split) re-shards ctx from
    head-major to token-major: core c ends with full-H ctx^T for tokens
    [b, 256c:256c+256) of each batch, then does its 1/8 of the output
    projection with the full Wo.

Pipeline structure: projections are emitted per 512-token quarter and
interleaved with the attention loops, so batch-0 attention starts as
soon as quarter 0 of D^T/V is done (~1MB of x loaded) instead of after
the full 8MB.  The attention inner loop is software-pipelined so the PE
never head-of-line blocks on ACT:
  PE:  score(kb,h0) score(kb,h1) ctx(kb-1,h0) ctx(kb-1,h1) score(kb+1)...
  ACT: exp(kb-1)    exp(kb)      ...   (one [128,1024] instr per kb,
                                        covering both heads)
Scores for both heads of one key block land in one 2-bank PSUM tile;
h0/h1 matmuls pack into disjoint PE row groups (K=64) resp. col groups
(ctx, M=64) and overlap.  Diagonal key blocks use ragged n (per-element
has_written handles the ragged accumulation) + a [128,128] tril mask
multiply on DVE.  Input DMA is split across two queues (sync/scalar);
Wo loads are deferred to overlap batch-1 attention.

Precision: x/Wd/dT fp16, V/ctx/at/out-proj bf16, PSUM fp32.
"""
import numpy as np
import ml_dtypes

import concourse.bass as bass
import concourse.bacc as bacc
import concourse.mybir as mybir
import concourse.tile as tile
from concourse.tile_rust import add_dep_helper
from concourse.bass_utils import run_bass_kernel_spmd

FP = mybir.dt.float32
F16 = mybir.dt.float16
BF = mybir.dt.bfloat16
AF = mybir.ActivationFunctionType

NC = 8
B, S, H, NH, HD = 2, 2048, 1024, 16, 64
T = B * S            # 4096 tokens
QB = 512             # query block
KB = 128             # key block
NQB = S // QB        # 4 query blocks per batch
NKB = S // KB        # 16 key blocks per batch

_cached = {}


def _ins(x):
    return getattr(x, "ins", x)


def _build():
    nc = bacc.Bacc("TRN2", target_bir_lowering=False, debug=False, num_devices=NC)

    xT = nc.dram_tensor("xT", [H, T], F16, kind="ExternalInput")
    WdT = nc.dram_tensor("WdT", [128, 1024], F16, kind="ExternalInput")
    WvT = nc.dram_tensor("WvT", [128, 1024], F16, kind="ExternalInput")
    WoT = nc.dram_tensor("WoT", [128, 8192], BF, kind="ExternalInput")
    bo_d = nc.dram_tensor("bo", [H], FP, kind="ExternalInput")
    mask_d = nc.dram_tensor("maskbf", [128, 128], BF, kind="ExternalInput")
    out_d = nc.dram_tensor("out", [H, 512], FP, kind="ExternalOutput")

    with tile.TileContext(nc) as tc:
        with (
            tc.tile_pool(name="res", bufs=1) as res,      # resident SBUF
            tc.tile_pool(name="stream", bufs=3) as strm,  # streamed SBUF
            tc.tile_pool(name="dram", bufs=1, space="DRAM") as dram,
            tc.tile_pool(name="psp", bufs=1, space="PSUM") as psp,
            tc.tile_pool(name="pssc", bufs=2, space="PSUM") as pssc,
            tc.tile_pool(name="pscx", bufs=1, space="PSUM") as pscx,
        ):
            # ---------------- constants / weights in ----------------
            # wd_t leads the sync queue (it gates the first D matmul);
            # mask/bo ride the gpsimd queue; wv is emitted after the first
            # x chunks (only needed once V(0) runs).
            wd_t = res.tile([128, 1024], F16, tag="wd")
            nc.sync.dma_start(wd_t[:], WdT[:])
            mask_t = res.tile([128, 128], BF, tag="mask")
            nc.gpsimd.dma_start(mask_t[:], mask_d[:])
            bo_t = res.tile([128, 8], FP, tag="bo")
            nc.gpsimd.dma_start(bo_t[:], bo_d[:].rearrange("(f p) -> p f", p=128))
            wv_t = res.tile([128, 1024], F16, tag="wv")
            # Wo allocated now, loaded later (only needed at out-proj)
            wo_t = res.tile([128, 8192], BF, tag="wo")

            # resident outputs of the projections (per 512-token quarter)
            dT = [res.tile([128, 512], F16, tag=f"dT{i}", name=f"dT{i}")
                  for i in range(8)]                  # D^T  [128 feat, 4096 tok]
            Vg = [res.tile([128, 512], BF, tag=f"Vg{i}", name=f"Vg{i}")
                  for i in range(8)]                  # V    [tok, feat] 4 tiles/grp

            xq_tiles = {}

            def emit_proj_d(q):
                """x loads + D^T for tokens [512q, 512q+512)."""
                xq = []
                engs = [nc.sync, nc.scalar, nc.gpsimd]
                for k in range(8):
                    x = strm.tile([128, 512], F16, tag="xq", bufs=32,
                                  name=f"xq{q}_{k}")
                    engs[k % 3].dma_start(
                        x[:], xT[k * 128:(k + 1) * 128,
                                 q * 512:(q + 1) * 512]
                    )
                    xq.append(x)
                xq_tiles[q] = xq
                pd = psp.tile([128, 512], FP, tag="pd", bufs=2, name=f"pd{q}")
                for k in range(8):
                    nc.tensor.matmul(
                        pd[:], wd_t[:, k * 128:(k + 1) * 128], xq[k][:],
                        start=(k == 0), stop=(k == 7),
                    )
                nc.vector.tensor_copy(dT[q][:], pd[:])

            def emit_proj_v(q):
                """V for tokens [512q, 512q+512)."""
                xq = xq_tiles.pop(q)
                pv = psp.tile([128, 512], FP, tag="pv", name=f"pv{q}")
                for k in range(8):
                    for t in range(4):                # 128-token tiles -> V
                        nc.tensor.matmul(
                            pv[:, t * 128:(t + 1) * 128],
                            xq[k][:, t * 128:(t + 1) * 128],
                            wv_t[:, k * 128:(k + 1) * 128],
                            start=(k == 0 and t == 0), stop=(k == 7),
                            skip_group_check=True,
                        )
                nc.vector.tensor_copy(Vg[q][:], pv[:])

            # ---------------- attention (local, 2 heads) ----------------
            # ctx^T accumulates into per-batch SBUF tiles [128, 2048] bf16
            ctxT = [res.tile([128, 2048], BF, tag=f"ctxT{b}", name=f"ctxT{b}")
                    for b in range(B)]

            ctxg = [None, None]                       # [128, 2048] bf16 per b

            def emit_outproj_group(b, fo, c0=0, cw=256):
                # shares the (2-bank) "ps" slots -> double-buffered out-proj;
                # cw=128 halves spread batch-0's groups across the A2A(b1)
                # window to keep the PE (HAM) warm
                po = pssc.tile([128, 512], FP, tag="ps", name=f"po{b}_{fo}_{c0}")
                first = None
                for k in range(8):
                    mm = nc.tensor.matmul(
                        po[:, 0:cw],
                        wo_t[:, k * 1024 + fo * 128:k * 1024 + (fo + 1) * 128],
                        ctxg[b][:, k * 256 + c0:k * 256 + c0 + cw],
                        start=(k == 0), stop=(k == 7),
                    )
                    if first is None:
                        first = mm
                ot = strm.tile([128, 256], FP, tag="ot", bufs=4,
                               name=f"ot{b}_{fo}_{c0}")
                nc.scalar.activation(
                    ot[:, 0:cw], po[:, 0:cw], AF.Identity,
                    bias=bo_t[:, fo:fo + 1], scale=1.0,
                )
                nc.sync.dma_start(
                    out_d[fo * 128:(fo + 1) * 128,
                          b * 256 + c0:b * 256 + c0 + cw],
                    ot[:, 0:cw],
                )
                return first

            attn_state = {}

            def emit_attn_qblock(b, qb, kb_lo=None, kb_hi=None):
                """Emit attention for query block (b, qb), key blocks
                [kb_lo, kb_hi).  May be called in consecutive chunks; the
                accumulator state is carried in attn_state."""
                qt = b * 4 + qb                       # query dT tile (512 tok)
                if kb_lo is None:
                    kb_lo = 4 * qb
                if kb_hi is None:
                    kb_hi = NKB
                if kb_lo == 4 * qb:
                    pc = pscx.tile([128, QB], FP, tag="pc", name=f"pc{b}_{qb}")
                    prev = []
                else:
                    pc, prev = attn_state[(b, qb)]
                for kb in range(kb_lo, kb_hi):
                    koff = b * S + kb * KB
                    kt, kc = koff // 512, koff % 512
                    n = min(QB, (kb + 1) * KB - qb * QB)
                    j = kb - 4 * qb                   # diag sub-block index
                    # -- scores for both heads into one 2-bank tile
                    ps = pssc.tile([128, 1024], FP, tag="ps",
                                   name=f"ps{b}_{qb}_{kb}")
                    for h in range(2):
                        nc.tensor.matmul(
                            ps[:, 512 * h:512 * h + n],
                            dT[kt][h * 64:(h + 1) * 64, kc:kc + 128],
                            dT[qt][h * 64:(h + 1) * 64, 0:n],
                            start=True, stop=True,
                        )
                    # -- exp on ACT (one wide instr per kb, both heads)
                    at = strm.tile([128, 1024], BF, tag="at", bufs=8,
                                   name=f"at{b}_{qb}_{kb}")
                    if n == QB:
                        nc.scalar.activation(
                            at[:], ps[:], AF.Exp, scale=-0.5
                        )
                    else:
                        nc.scalar.activation(
                            at[:].rearrange("p (h c) -> p h c", h=2)[:, :, 0:n],
                            ps[:].rearrange("p (h c) -> p h c", h=2)[:, :, 0:n],
                            AF.Exp, scale=-0.5,
                        )
                    if j < 4:                         # diagonal: tril mask on
                        for h in range(2):            # the (idle) gpsimd engine
                            lo = 512 * h + 128 * j
                            nc.gpsimd.tensor_mul(
                                at[:, lo:lo + 128],
                                at[:, lo:lo + 128], mask_t[:]
                            )
                    # -- ctx lags the score stream; emitted two-at-a-time so
                    # the PE sees longer bursts (s,s,c,c -- fewer micro-idle
                    # boundaries for the HAM clock gate)
                    prev.append((at, n, (b * 16 + kb) // 4, (b * 16 + kb) % 4,
                                 kb == 4 * qb, kb == NKB - 1))
                    if len(prev) > 3:
                        _emit_ctx(nc, pc, Vg, prev.pop(0))
                        _emit_ctx(nc, pc, Vg, prev.pop(0))
                if kb_hi < NKB:
                    attn_state[(b, qb)] = (pc, prev)
                    return None
                last = None
                while prev:
                    last = _emit_ctx(nc, pc, Vg, prev.pop(0))
                nc.vector.tensor_copy(
                    ctxT[b][:, qb * QB:(qb + 1) * QB], pc[:]
                )
                emit_stage(b, qb)
                return last

            cc_ins = [dram.tile([1024, 256], BF, name=f"cc_in{b}")
                      for b in range(B)]

            def emit_stage(b, qb):
                # stage this qb's two complete slabs into the A2A input as
                # soon as they are written (slab j = tokens [256j, 256j+256))
                cc_in = cc_ins[b]
                nc.sync.dma_start(
                    cc_in[256 * qb:256 * (qb + 1), :]
                    .rearrange("(j p) c -> p j c", p=128),
                    ctxT[b][:, 512 * qb:512 * (qb + 1)]
                    .rearrange("p (j c) -> p j c", j=2),
                )

            def emit_a2a(b):
                cc_in = cc_ins[b]
                cc_out = dram.tile([1024, 256], BF, name=f"cc_out{b}")
                nc.gpsimd.collective_compute(
                    "AllToAll",
                    mybir.AluOpType.bypass,
                    replica_groups=[list(range(NC))],
                    ins=[cc_in[:].opt()],
                    outs=[cc_out[:].opt()],
                )
                # per-k loads: the first out-proj matmul can start after one
                # 64KB chunk lands instead of the whole 512KB
                g = res.tile([128, 2048], BF, tag=f"cg{b}", name=f"cg{b}")
                for k in range(8):
                    nc.sync.dma_start(
                        g[:, k * 256:(k + 1) * 256],
                        cc_out[k * 128:(k + 1) * 128, :],
                    )
                ctxg[b] = g

            # warm the collective stream early so the first real A2A does
            # not pay the ~11us cold trigger-start delay
            cc_wu_in = dram.tile([1024, 8], BF, name="cc_wu_in")
            cc_wu_out = dram.tile([1024, 8], BF, name="cc_wu_out")
            nc.gpsimd.collective_compute(
                "AllToAll",
                mybir.AluOpType.bypass,
                replica_groups=[list(range(NC))],
                ins=[cc_wu_in[:].opt()],
                outs=[cc_wu_out[:].opt()],
            )

            # quarter-granular projections interleaved with attention:
            # b0-qb blocks need dT/Vg[0..3]; b1 needs [4..7].  D parts are
            # emitted ahead of V parts so the exp stream isn't starved;
            # attention emission starts as early as its inputs allow, and
            # later proj parts fill PE slack via priority.
            emit_proj_d(0)
            nc.scalar.dma_start(wv_t[:], WvT[:])
            emit_proj_v(0)
            emit_attn_qblock(0, 0, 0, 4)
            emit_proj_d(1)
            emit_proj_v(1)
            emit_attn_qblock(0, 0, 4, 8)
            emit_proj_d(2)
            emit_proj_v(2)
            emit_proj_d(3)
            emit_proj_v(3)
            emit_attn_qblock(0, 0, 8, NKB)
            emit_attn_qblock(0, 1)
            emit_proj_d(4)
            emit_proj_v(4)
            emit_attn_qblock(0, 2)
            emit_proj_d(5)
            emit_proj_v(5)
            emit_attn_qblock(0, 3)
            # A2A(b0) is emitted first: its gpsimd trigger only waits on
            # b0's staging, and emitting it later would trap it behind
            # b1's mask-muls in the in-order gpsimd queue.
            emit_a2a(0)
            # b1-qb0 needs only dT/Vg[4 + kb//4] per key block, so its kb
            # chunks interleave with the last two proj quarters -- the exp
            # stream no longer starves while the PE finishes q6/q7.
            emit_attn_qblock(1, 0, 0, 4)
            emit_proj_d(6)
            emit_proj_v(6)
            emit_attn_qblock(1, 0, 4, 8)
            emit_proj_d(7)
            emit_proj_v(7)
            # deferred Wo load drains in DMA-queue slack from here on
            nc.scalar.dma_start(wo_t[:], WoT[:])
            emit_attn_qblock(1, 0, 8, NKB)

            last_b1 = None
            for qb in range(1, NQB):
                last_b1 = emit_attn_qblock(1, qb)
            emit_a2a(1)

            # batch-0 output projection overlaps A2A(b1); the explicit dep
            # keeps it from preempting b1 attention at the PE queue head.
            # Half-width groups spread it across the whole A2A window so
            # the PE stays warm for batch-1's projection.
            first = emit_outproj_group(0, 0, 0, 128)
            add_dep_helper(
                _ins(first), _ins(last_b1), sync=False,
                reason="outproj(b0) after b1 attention",
            )
            emit_outproj_group(0, 0, 128, 128)
            for fo in range(1, 8):
                emit_outproj_group(0, fo, 0, 128)
                emit_outproj_group(0, fo, 128, 128)
            for fo in range(8):
                emit_outproj_group(1, fo)

    nc.compile()
    return nc


def _emit_ctx(nc, pc, Vg, prev):
    at, n, g, m, first, last = prev
    mm = None
    for h in range(2):
        mm = nc.tensor.matmul(
            pc[h * 64:(h + 1) * 64, 0:n],
            Vg[g][:, m * 128 + h * 64:m * 128 + h * 64 + 64],
            at[:, 512 * h:512 * h + n],
            start=first, stop=last,
            skip_group_check=True,
        )
    return mm


def make_in_maps(inputs):
    x = np.asarray(inputs["x"], np.float32)
    Wq = np.asarray(inputs["Wq"], np.float32)
    Wk = np.asarray(inputs["Wk"], np.float32)
    Wv = np.asarray(inputs["Wv"], np.float32)
    Wo = np.asarray(inputs["Wo"], np.float32)
    bo = np.asarray(inputs["bo"], np.float32)
    # bq/bk/bv are zeros by the problem's input spec; dif = x @ (Wq - Wk)
    # and v = x @ Wv absorb them exactly when zero.

    xT = np.ascontiguousarray(x.reshape(T, H).T).astype(np.float16)
    Wd = (Wq - Wk).astype(np.float16)
    # pre-transpose to SBUF layout: tile[p, 128k + m] = W[128k + p, m]
    WoT = np.ascontiguousarray(
        Wo.reshape(8, 128, 1024).transpose(1, 0, 2).reshape(128, 8192)
    ).astype(ml_dtypes.bfloat16)
    maskbf = np.tril(np.ones((128, 128), np.float32), -1).astype(ml_dtypes.bfloat16)

    in_maps = []
    for c in range(NC):
        cols = slice(c * 128, (c + 1) * 128)
        WdT = np.ascontiguousarray(
            Wd[:, cols].reshape(8, 128, 128).transpose(1, 0, 2).reshape(128, 1024)
        )
        WvT = np.ascontiguousarray(
            Wv[:, cols].astype(np.float16)
            .reshape(8, 128, 128).transpose(1, 0, 2).reshape(128, 1024)
        )
        in_maps.append({
            "xT": xT,
            "WdT": WdT,
            "WvT": WvT,
            "WoT": WoT,
            "bo": bo,
            "maskbf": maskbf,
        })
    return in_maps


def kernel(**inputs):
    if "nc" not in _cached:
        _cached["nc"] = _build()
    nc = _cached["nc"]

    res = run_bass_kernel_spmd(nc, make_in_maps(inputs), core_ids=list(range(NC)))

    out = np.empty((B, S, H), np.float32)
    for c in range(NC):
        oT = res.results[c]["out"]                    # [H, 512]
        for b in range(B):
            out[b, c * 256:(c + 1) * 256, :] = oT[:, b * 256:(b + 1) * 256].T
    return out
